# revision 6
# baseline (speedup 1.0000x reference)
"""TRN2 Bass kernel for the LSQ-quantized 2-layer MLP.

reference computation:
    wq1 = lsq_quant(w1, alpha1); wq2 = lsq_quant(w2, alpha2)   (tiny 256x256)
    h = relu(x @ wq1.T + b1)
    y = sigmoid(h @ wq2.T + b2)                                 x: [262144, 256] f32

Data-parallel over 8 NeuronCores (32768 tokens/core). Host-side prep per
shard: transpose x to channel-major (so the contraction dim lands on SBUF
partitions with plain DMAs — no on-chip transposes), and split the LSQ
quantization into integer levels k = round(clip(w/a, -8, 7)) (exactly
representable in bf16) and the scale a, which is folded into the activation
(relu(a*z+b1), sigmoid(a*z+b2)). Matmuls run in bf16 with fp32 PSUM
accumulation; the only precision loss is the bf16 rounding of x and h.

Everything is computed in the transposed (channel-major) domain:
    SWDGE cast-DMA xT tile (f32 DRAM -> bf16 SBUF)
    -> fc1: 4 matmuls (w1 chunks stationary, xT moving, N=512)  -> hT PSUM
    -> relu(a1*z [+ b1]) on DVE (ACT when b1 != 0)              -> bf16 SBUF
    -> fc2: 4 matmuls (w2 chunks stationary, hT moving, N=512)  -> yT PSUM
    -> sigmoid(a2*z [+ b2]) on ACT (b2 is per-partition here)   -> f32 SBUF
    -> DMA out yT (ACT queue; host un-transposes at gather)
"""

import ml_dtypes
import numpy as np

import concourse.bass as bass
import concourse.mybir as mybir
import concourse.tile as tile
from concourse import bacc
from concourse.bass import ts
from concourse.bass_utils import run_bass_kernel_spmd

N_CORES = 8
N_TOK = 262144
C = 256
TOK_PER_CORE = N_TOK // N_CORES  # 32768
T_MACRO = 512
N_MACROS = TOK_PER_CORE // T_MACRO  # 64
P = 128

F32 = mybir.dt.float32
BF16 = mybir.dt.bfloat16

_program_cache = {}


def _build_program(use_b1: bool, use_b2: bool):
    nc = bacc.Bacc("TRN2", target_bir_lowering=False, debug=False, num_devices=N_CORES)

    xt_d = nc.declare_dram_parameter("xt", [C, TOK_PER_CORE], F32, isOutput=False)
    w1k_d = nc.declare_dram_parameter("w1k", [P, 2, C], BF16, isOutput=False)
    w2k_d = nc.declare_dram_parameter("w2k", [P, 2, C], BF16, isOutput=False)
    a1_d = nc.declare_dram_parameter("a1", [P, 1], F32, isOutput=False)
    a2_d = nc.declare_dram_parameter("a2", [P, 1], F32, isOutput=False)
    if use_b1:
        b1s_d = nc.declare_dram_parameter("b1s", [P, 2], F32, isOutput=False)
    if use_b2:
        b2s_d = nc.declare_dram_parameter("b2s", [P, 2], F32, isOutput=False)
    yt_d = nc.declare_dram_parameter("yt", [C, TOK_PER_CORE], F32, isOutput=True)

    xt_v = xt_d.rearrange("(co ci) (m t) -> m ci co t", ci=P, t=T_MACRO)
    yt_v = yt_d.rearrange("(co ci) (m t) -> m ci co t", ci=P, t=T_MACRO)

    with tile.TileContext(nc) as tc:
        with (
            tc.tile_pool(name="const", bufs=1) as const_pool,
            tc.tile_pool(name="sb_xt", bufs=4) as sb_xt,
            tc.tile_pool(name="sb_ht", bufs=4) as sb_ht,
            tc.tile_pool(name="sb_yt", bufs=4) as sb_yt,
            tc.tile_pool(name="ps_h", bufs=4, space="PSUM") as ps_h,
            tc.tile_pool(name="ps_y", bufs=4, space="PSUM") as ps_y,
        ):
            w1k = const_pool.tile([P, 2, C], BF16)
            w2k = const_pool.tile([P, 2, C], BF16)
            nc.sync.dma_start(w1k[:], w1k_d[:])
            nc.sync.dma_start(w2k[:], w2k_d[:])
            a1 = const_pool.tile([P, 1], F32)
            a2 = const_pool.tile([P, 1], F32)
            nc.sync.dma_start(a1[:], a1_d[:])
            nc.sync.dma_start(a2[:], a2_d[:])
            if use_b1:
                b1s = const_pool.tile([P, 2], F32)
                nc.sync.dma_start(b1s[:], b1s_d[:])
            if use_b2:
                b2s = const_pool.tile([P, 2], F32)
                nc.sync.dma_start(b2s[:], b2s_d[:])

            for m in range(N_MACROS):
                # f32 DRAM -> bf16 SBUF cast during the DMA (SWDGE)
                xt = sb_xt.tile([P, 2, T_MACRO], BF16, tag="xt")
                nc.gpsimd.dma_start(xt[:], xt_v[m])

                # fc1: hT[j_chunk] = sum_c w1k[:,c,jchunk].T @ xT[:,c,:]
                ht = sb_ht.tile([P, 2, T_MACRO], BF16, tag="ht")
                for j in range(2):
                    pht = ps_h.tile([P, T_MACRO], F32, tag="pht")
                    for c in range(2):
                        nc.tensor.matmul(
                            pht[:],
                            w1k[:, c, ts(j, P)],
                            xt[:, c, :],
                            start=(c == 0),
                            stop=(c == 1),
                        )
                    if use_b1:
                        nc.scalar.activation(
                            ht[:, j, :],
                            pht[:],
                            mybir.ActivationFunctionType.Relu,
                            bias=b1s[:, j : j + 1],
                            scale=a1[:],
                        )
                    else:
                        # relu(a1*z) on DVE: (z * a1) max 0
                        nc.vector.tensor_scalar(
                            ht[:, j, :],
                            pht[:],
                            a1[:],
                            0.0,
                            mybir.AluOpType.mult,
                            mybir.AluOpType.max,
                        )

                # fc2: yT[j_chunk] = sum_c w2k[:,c,jchunk].T @ hT[:,c,:]
                yt = sb_yt.tile([P, 2, T_MACRO], F32, tag="yt")
                for j in range(2):
                    pyt = ps_y.tile([P, T_MACRO], F32, tag="pyt")
                    for c in range(2):
                        nc.tensor.matmul(
                            pyt[:],
                            w2k[:, c, ts(j, P)],
                            ht[:, c, :],
                            start=(c == 0),
                            stop=(c == 1),
                        )
                    nc.scalar.activation(
                        yt[:, j, :],
                        pyt[:],
                        mybir.ActivationFunctionType.Sigmoid,
                        bias=b2s[:, j : j + 1] if use_b2 else 0.0,
                        scale=a2[:],
                    )
                nc.scalar.dma_start(yt_v[m], yt[:])

    nc.compile()
    return nc


def _quantize_lsq_int(w: np.ndarray, alpha) -> tuple[np.ndarray, np.float32]:
    """Integer LSQ levels k = round(clip(w/a, -8, 7)) and effective scale a,
    replicating the reference forward numerics in np float32."""
    one = np.float32(1.0)
    g = one / np.sqrt(np.float32(w.size * 7))
    alpha = np.float32(alpha)
    a = np.float32(alpha * g) + np.float32(alpha * np.float32(one - g))
    t = np.clip((w / a).astype(np.float32), np.float32(-8.0), np.float32(7.0))
    r = (np.round(t) - t).astype(np.float32)
    q = (t + r).astype(np.float32)  # integer levels in [-8, 7]
    return q, a


def _prepare(x, w1, b1, alpha1, w2, b2, alpha2):
    x = np.asarray(x, dtype=np.float32)
    w1 = np.asarray(w1, dtype=np.float32)
    w2 = np.asarray(w2, dtype=np.float32)
    b1 = np.asarray(b1, dtype=np.float32)
    b2 = np.asarray(b2, dtype=np.float32)

    k1, a1 = _quantize_lsq_int(w1, alpha1)
    k2, a2 = _quantize_lsq_int(w2, alpha2)

    # lhsT layouts: w1k[ci, co, j] = k1[j, co*128+ci]
    w1k = np.ascontiguousarray(k1.T.reshape(2, P, C).transpose(1, 0, 2)).astype(
        ml_dtypes.bfloat16
    )
    w2k = np.ascontiguousarray(k2.T.reshape(2, P, C).transpose(1, 0, 2)).astype(
        ml_dtypes.bfloat16
    )

    use_b1 = bool(np.any(b1))
    use_b2 = bool(np.any(b2))
    key = (use_b1, use_b2)
    if key not in _program_cache:
        _program_cache[key] = _build_program(use_b1, use_b2)
    nc = _program_cache[key]

    a1_col = np.full((P, 1), a1, dtype=np.float32)
    a2_col = np.full((P, 1), a2, dtype=np.float32)

    in_maps = []
    for i in range(N_CORES):
        shard = x[i * TOK_PER_CORE : (i + 1) * TOK_PER_CORE]
        m = {
            "xt": np.ascontiguousarray(shard.T),
            "w1k": w1k,
            "w2k": w2k,
            "a1": a1_col,
            "a2": a2_col,
        }
        if use_b1:
            m["b1s"] = np.ascontiguousarray(b1.reshape(2, P).T)
        if use_b2:
            m["b2s"] = np.ascontiguousarray(b2.reshape(2, P).T)
        in_maps.append(m)
    return nc, in_maps


def kernel(x, w1, b1, alpha1, w2, b2, alpha2):
    nc, in_maps = _prepare(x, w1, b1, alpha1, w2, b2, alpha2)
    res = run_bass_kernel_spmd(nc, in_maps, list(range(N_CORES)))
    out = np.concatenate(
        [np.ascontiguousarray(res.results[i]["yt"].T) for i in range(N_CORES)],
        axis=0,
    )
    return out


# revision 8
# speedup vs baseline: 1.2200x; 1.2200x over previous
"""TRN2 Bass kernel for the LSQ-quantized 2-layer MLP.

reference computation:
    wq1 = lsq_quant(w1, alpha1); wq2 = lsq_quant(w2, alpha2)   (tiny 256x256)
    h = relu(x @ wq1.T + b1)
    y = sigmoid(h @ wq2.T + b2)                                 x: [262144, 256] f32

Data-parallel over 8 NeuronCores (32768 tokens/core). Host-side prep per
shard: transpose x to channel-major (so the contraction dim lands on SBUF
partitions with plain DMAs — no on-chip transposes), and split the LSQ
quantization into integer levels k = round(clip(w/a, -8, 7)) (exactly
representable in bf16) and the scale a, which is folded into the activation
(relu(a*z+b1), sigmoid(a*z+b2)). Matmuls run in bf16 with fp32 PSUM
accumulation; the only precision loss is the bf16 rounding of x and h.

Everything is computed in the transposed (channel-major) domain:
    SWDGE cast-DMA xT tile (f32 DRAM -> bf16 SBUF)
    -> fc1: 4 matmuls (w1 chunks stationary, xT moving, N=512)  -> hT PSUM
    -> relu(a1*z [+ b1]) on DVE (ACT when b1 != 0)              -> bf16 SBUF
    -> fc2: 4 matmuls (w2 chunks stationary, hT moving, N=512)  -> yT PSUM
    -> sigmoid(a2*z [+ b2]) on ACT (b2 is per-partition here)   -> f32 SBUF
    -> DMA out yT (ACT queue; host un-transposes at gather)
"""

import ml_dtypes
import numpy as np

import concourse.bass as bass
import concourse.mybir as mybir
import concourse.tile as tile
from concourse import bacc
from concourse.bass import ts
from concourse.bass_utils import run_bass_kernel_spmd

N_CORES = 8
N_TOK = 262144
C = 256
TOK_PER_CORE = N_TOK // N_CORES  # 32768
T_MACRO = 512
N_MACROS = TOK_PER_CORE // T_MACRO  # 64
P = 128

F32 = mybir.dt.float32
BF16 = mybir.dt.bfloat16

_program_cache = {}


def _build_program(use_b1: bool, use_b2: bool):
    nc = bacc.Bacc("TRN2", target_bir_lowering=False, debug=False, num_devices=N_CORES)

    xt_d = nc.declare_dram_parameter("xt", [C, TOK_PER_CORE], F32, isOutput=False)
    w1k_d = nc.declare_dram_parameter("w1k", [P, 2, C], BF16, isOutput=False)
    w2k_d = nc.declare_dram_parameter("w2k", [P, 2, C], BF16, isOutput=False)
    a1_d = nc.declare_dram_parameter("a1", [P, 1], F32, isOutput=False)
    a2_d = nc.declare_dram_parameter("a2", [P, 1], F32, isOutput=False)
    if use_b1:
        b1s_d = nc.declare_dram_parameter("b1s", [P, 2], F32, isOutput=False)
    if use_b2:
        b2s_d = nc.declare_dram_parameter("b2s", [P, 2], F32, isOutput=False)
    yt_d = nc.declare_dram_parameter("yt", [C, TOK_PER_CORE], F32, isOutput=True)

    # 1024-token super-macros: one 1 MiB load / store per pair of compute macros
    xt_v = xt_d.rearrange("(co ci) (m t) -> m ci co t", ci=P, t=2 * T_MACRO)
    yt_v = yt_d.rearrange("(co ci) (m t) -> m ci co t", ci=P, t=2 * T_MACRO)

    with tile.TileContext(nc) as tc:
        with (
            tc.tile_pool(name="const", bufs=1) as const_pool,
            tc.tile_pool(name="sb_xt", bufs=4) as sb_xt,
            tc.tile_pool(name="sb_ht", bufs=4) as sb_ht,
            tc.tile_pool(name="sb_yt", bufs=4) as sb_yt,
            tc.tile_pool(name="ps_h", bufs=4, space="PSUM") as ps_h,
            tc.tile_pool(name="ps_y", bufs=4, space="PSUM") as ps_y,
        ):
            w1k = const_pool.tile([P, 2, C], BF16)
            w2k = const_pool.tile([P, 2, C], BF16)
            nc.sync.dma_start(w1k[:], w1k_d[:])
            nc.sync.dma_start(w2k[:], w2k_d[:])
            a1 = const_pool.tile([P, 1], F32)
            a2 = const_pool.tile([P, 1], F32)
            nc.sync.dma_start(a1[:], a1_d[:])
            nc.sync.dma_start(a2[:], a2_d[:])
            if use_b1:
                b1s = const_pool.tile([P, 2], F32)
                nc.sync.dma_start(b1s[:], b1s_d[:])
            if use_b2:
                b2s = const_pool.tile([P, 2], F32)
                nc.sync.dma_start(b2s[:], b2s_d[:])

            for m in range(N_MACROS // 2):
                # f32 DRAM -> bf16 SBUF cast during the DMA (SWDGE), 1 MiB read
                xt = sb_xt.tile([P, 2, 2 * T_MACRO], BF16, tag="xt")
                nc.gpsimd.dma_start(xt[:], xt_v[m])

                yt = sb_yt.tile([P, 2, 2 * T_MACRO], F32, tag="yt")
                for s in range(2):
                    tok = ts(s, T_MACRO)
                    # fc1: hT[j_chunk] = sum_c w1k[:,c,jchunk].T @ xT[:,c,:]
                    ht = sb_ht.tile([P, 2, T_MACRO], BF16, tag="ht")
                    for j in range(2):
                        pht = ps_h.tile([P, T_MACRO], F32, tag="pht")
                        for c in range(2):
                            nc.tensor.matmul(
                                pht[:],
                                w1k[:, c, ts(j, P)],
                                xt[:, c, tok],
                                start=(c == 0),
                                stop=(c == 1),
                            )
                        if use_b1:
                            nc.scalar.activation(
                                ht[:, j, :],
                                pht[:],
                                mybir.ActivationFunctionType.Relu,
                                bias=b1s[:, j : j + 1],
                                scale=a1[:],
                            )
                        else:
                            # relu(a1*z) on DVE: (z * a1) max 0
                            nc.vector.tensor_scalar(
                                ht[:, j, :],
                                pht[:],
                                a1[:],
                                0.0,
                                mybir.AluOpType.mult,
                                mybir.AluOpType.max,
                            )

                    # fc2: yT[j_chunk] = sum_c w2k[:,c,jchunk].T @ hT[:,c,:]
                    for j in range(2):
                        pyt = ps_y.tile([P, T_MACRO], F32, tag="pyt")
                        for c in range(2):
                            nc.tensor.matmul(
                                pyt[:],
                                w2k[:, c, ts(j, P)],
                                ht[:, c, :],
                                start=(c == 0),
                                stop=(c == 1),
                            )
                        nc.scalar.activation(
                            yt[:, j, tok],
                            pyt[:],
                            mybir.ActivationFunctionType.Sigmoid,
                            bias=b2s[:, j : j + 1] if use_b2 else 0.0,
                            scale=a2[:],
                        )
                nc.scalar.dma_start(yt_v[m], yt[:])

    nc.compile()
    return nc


def _quantize_lsq_int(w: np.ndarray, alpha) -> tuple[np.ndarray, np.float32]:
    """Integer LSQ levels k = round(clip(w/a, -8, 7)) and effective scale a,
    replicating the reference forward numerics in np float32."""
    one = np.float32(1.0)
    g = one / np.sqrt(np.float32(w.size * 7))
    alpha = np.float32(alpha)
    a = np.float32(alpha * g) + np.float32(alpha * np.float32(one - g))
    t = np.clip((w / a).astype(np.float32), np.float32(-8.0), np.float32(7.0))
    r = (np.round(t) - t).astype(np.float32)
    q = (t + r).astype(np.float32)  # integer levels in [-8, 7]
    return q, a


def _prepare(x, w1, b1, alpha1, w2, b2, alpha2):
    x = np.asarray(x, dtype=np.float32)
    w1 = np.asarray(w1, dtype=np.float32)
    w2 = np.asarray(w2, dtype=np.float32)
    b1 = np.asarray(b1, dtype=np.float32)
    b2 = np.asarray(b2, dtype=np.float32)

    k1, a1 = _quantize_lsq_int(w1, alpha1)
    k2, a2 = _quantize_lsq_int(w2, alpha2)

    # lhsT layouts: w1k[ci, co, j] = k1[j, co*128+ci]
    w1k = np.ascontiguousarray(k1.T.reshape(2, P, C).transpose(1, 0, 2)).astype(
        ml_dtypes.bfloat16
    )
    w2k = np.ascontiguousarray(k2.T.reshape(2, P, C).transpose(1, 0, 2)).astype(
        ml_dtypes.bfloat16
    )

    use_b1 = bool(np.any(b1))
    use_b2 = bool(np.any(b2))
    key = (use_b1, use_b2)
    if key not in _program_cache:
        _program_cache[key] = _build_program(use_b1, use_b2)
    nc = _program_cache[key]

    a1_col = np.full((P, 1), a1, dtype=np.float32)
    a2_col = np.full((P, 1), a2, dtype=np.float32)

    in_maps = []
    for i in range(N_CORES):
        shard = x[i * TOK_PER_CORE : (i + 1) * TOK_PER_CORE]
        m = {
            "xt": np.ascontiguousarray(shard.T),
            "w1k": w1k,
            "w2k": w2k,
            "a1": a1_col,
            "a2": a2_col,
        }
        if use_b1:
            m["b1s"] = np.ascontiguousarray(b1.reshape(2, P).T)
        if use_b2:
            m["b2s"] = np.ascontiguousarray(b2.reshape(2, P).T)
        in_maps.append(m)
    return nc, in_maps


def kernel(x, w1, b1, alpha1, w2, b2, alpha2):
    nc, in_maps = _prepare(x, w1, b1, alpha1, w2, b2, alpha2)
    res = run_bass_kernel_spmd(nc, in_maps, list(range(N_CORES)))
    out = np.concatenate(
        [np.ascontiguousarray(res.results[i]["yt"].T) for i in range(N_CORES)],
        axis=0,
    )
    return out


# revision 9
# speedup vs baseline: 1.5489x; 1.2696x over previous
"""TRN2 Bass kernel for the LSQ-quantized 2-layer MLP.

reference computation:
    wq1 = lsq_quant(w1, alpha1); wq2 = lsq_quant(w2, alpha2)   (tiny 256x256)
    h = relu(x @ wq1.T + b1)
    y = sigmoid(h @ wq2.T + b2)                                 x: [262144, 256] f32

Data-parallel over 8 NeuronCores (32768 tokens/core). Host-side prep per
shard: transpose x to channel-major (so the contraction dim lands on SBUF
partitions with plain DMAs — no on-chip transposes), and split the LSQ
quantization into integer levels k = round(clip(w/a, -8, 7)) (exactly
representable in bf16) and the scale a, which is folded into the activation
(relu(a*z+b1), sigmoid(a*z+b2)). Matmuls run in bf16 with fp32 PSUM
accumulation; the only precision loss is the bf16 rounding of x and h.

Everything is computed in the transposed (channel-major) domain:
    SWDGE cast-DMA xT tile (f32 DRAM -> bf16 SBUF)
    -> fc1: 4 matmuls (w1 chunks stationary, xT moving, N=512)  -> hT PSUM
    -> relu(a1*z [+ b1]) on DVE (ACT when b1 != 0)              -> bf16 SBUF
    -> fc2: 4 matmuls (w2 chunks stationary, hT moving, N=512)  -> yT PSUM
    -> sigmoid(a2*z [+ b2]) on ACT (b2 is per-partition here)   -> f32 SBUF
    -> DMA out yT (ACT queue; host un-transposes at gather)
"""

import ml_dtypes
import numpy as np

import concourse.bass as bass
import concourse.mybir as mybir
import concourse.tile as tile
from concourse import bacc
from concourse.bass import ts
from concourse.bass_utils import run_bass_kernel_spmd

N_CORES = 8
N_TOK = 262144
C = 256
TOK_PER_CORE = N_TOK // N_CORES  # 32768
T_MACRO = 512
N_MACROS = TOK_PER_CORE // T_MACRO  # 64
P = 128

F32 = mybir.dt.float32
F16 = mybir.dt.float16
BF16 = mybir.dt.bfloat16

_program_cache = {}


def _build_program(use_b1: bool, use_b2: bool):
    nc = bacc.Bacc("TRN2", target_bir_lowering=False, debug=False, num_devices=N_CORES)

    xt_d = nc.declare_dram_parameter("xt", [C, TOK_PER_CORE], F32, isOutput=False)
    w1k_d = nc.declare_dram_parameter("w1k", [P, 2, C], BF16, isOutput=False)
    w2k_d = nc.declare_dram_parameter("w2k", [P, 2, C], BF16, isOutput=False)
    a1_d = nc.declare_dram_parameter("a1", [P, 1], F32, isOutput=False)
    a2_d = nc.declare_dram_parameter("a2", [P, 1], F32, isOutput=False)
    if use_b1:
        b1s_d = nc.declare_dram_parameter("b1s", [P, 2], F32, isOutput=False)
    if use_b2:
        b2s_d = nc.declare_dram_parameter("b2s", [P, 2], F32, isOutput=False)
    yt_d = nc.declare_dram_parameter("yt", [C, TOK_PER_CORE], F16, isOutput=True)

    # 1024-token super-macros: one 1 MiB load / store per pair of compute macros
    xt_v = xt_d.rearrange("(co ci) (m t) -> m ci co t", ci=P, t=2 * T_MACRO)
    yt_v = yt_d.rearrange("(co ci) (m t) -> m ci co t", ci=P, t=2 * T_MACRO)

    with tile.TileContext(nc) as tc:
        with (
            tc.tile_pool(name="const", bufs=1) as const_pool,
            tc.tile_pool(name="sb_xt", bufs=4) as sb_xt,
            tc.tile_pool(name="sb_ht", bufs=4) as sb_ht,
            tc.tile_pool(name="sb_yt", bufs=4) as sb_yt,
            tc.tile_pool(name="ps_h", bufs=4, space="PSUM") as ps_h,
            tc.tile_pool(name="ps_y", bufs=4, space="PSUM") as ps_y,
        ):
            w1k = const_pool.tile([P, 2, C], BF16)
            w2k = const_pool.tile([P, 2, C], BF16)
            nc.sync.dma_start(w1k[:], w1k_d[:])
            nc.sync.dma_start(w2k[:], w2k_d[:])
            a1 = const_pool.tile([P, 1], F32)
            a2 = const_pool.tile([P, 1], F32)
            nc.sync.dma_start(a1[:], a1_d[:])
            nc.sync.dma_start(a2[:], a2_d[:])
            if use_b1:
                b1s = const_pool.tile([P, 2], F32)
                nc.sync.dma_start(b1s[:], b1s_d[:])
            if use_b2:
                b2s = const_pool.tile([P, 2], F32)
                nc.sync.dma_start(b2s[:], b2s_d[:])

            for m in range(N_MACROS // 2):
                # f32 DRAM -> bf16 SBUF cast during the DMA (SWDGE), 1 MiB read
                xt = sb_xt.tile([P, 2, 2 * T_MACRO], BF16, tag="xt")
                nc.gpsimd.dma_start(xt[:], xt_v[m])

                yt = sb_yt.tile([P, 2, 2 * T_MACRO], F16, tag="yt")
                for s in range(2):
                    tok = ts(s, T_MACRO)
                    # fc1: hT[j_chunk] = sum_c w1k[:,c,jchunk].T @ xT[:,c,:]
                    ht = sb_ht.tile([P, 2, T_MACRO], BF16, tag="ht")
                    for j in range(2):
                        pht = ps_h.tile([P, T_MACRO], F32, tag="pht")
                        for c in range(2):
                            nc.tensor.matmul(
                                pht[:],
                                w1k[:, c, ts(j, P)],
                                xt[:, c, tok],
                                start=(c == 0),
                                stop=(c == 1),
                            )
                        if use_b1:
                            nc.scalar.activation(
                                ht[:, j, :],
                                pht[:],
                                mybir.ActivationFunctionType.Relu,
                                bias=b1s[:, j : j + 1],
                                scale=a1[:],
                            )
                        else:
                            # relu(a1*z) on DVE: (z * a1) max 0
                            nc.vector.tensor_scalar(
                                ht[:, j, :],
                                pht[:],
                                a1[:],
                                0.0,
                                mybir.AluOpType.mult,
                                mybir.AluOpType.max,
                            )

                    # fc2: yT[j_chunk] = sum_c w2k[:,c,jchunk].T @ hT[:,c,:]
                    for j in range(2):
                        pyt = ps_y.tile([P, T_MACRO], F32, tag="pyt")
                        for c in range(2):
                            nc.tensor.matmul(
                                pyt[:],
                                w2k[:, c, ts(j, P)],
                                ht[:, c, :],
                                start=(c == 0),
                                stop=(c == 1),
                            )
                        nc.scalar.activation(
                            yt[:, j, tok],
                            pyt[:],
                            mybir.ActivationFunctionType.Sigmoid,
                            bias=b2s[:, j : j + 1] if use_b2 else 0.0,
                            scale=a2[:],
                        )
                nc.sync.dma_start(yt_v[m], yt[:])

    nc.compile()
    return nc


def _quantize_lsq_int(w: np.ndarray, alpha) -> tuple[np.ndarray, np.float32]:
    """Integer LSQ levels k = round(clip(w/a, -8, 7)) and effective scale a,
    replicating the reference forward numerics in np float32."""
    one = np.float32(1.0)
    g = one / np.sqrt(np.float32(w.size * 7))
    alpha = np.float32(alpha)
    a = np.float32(alpha * g) + np.float32(alpha * np.float32(one - g))
    t = np.clip((w / a).astype(np.float32), np.float32(-8.0), np.float32(7.0))
    r = (np.round(t) - t).astype(np.float32)
    q = (t + r).astype(np.float32)  # integer levels in [-8, 7]
    return q, a


def _prepare(x, w1, b1, alpha1, w2, b2, alpha2):
    x = np.asarray(x, dtype=np.float32)
    w1 = np.asarray(w1, dtype=np.float32)
    w2 = np.asarray(w2, dtype=np.float32)
    b1 = np.asarray(b1, dtype=np.float32)
    b2 = np.asarray(b2, dtype=np.float32)

    k1, a1 = _quantize_lsq_int(w1, alpha1)
    k2, a2 = _quantize_lsq_int(w2, alpha2)

    # lhsT layouts: w1k[ci, co, j] = k1[j, co*128+ci]
    w1k = np.ascontiguousarray(k1.T.reshape(2, P, C).transpose(1, 0, 2)).astype(
        ml_dtypes.bfloat16
    )
    w2k = np.ascontiguousarray(k2.T.reshape(2, P, C).transpose(1, 0, 2)).astype(
        ml_dtypes.bfloat16
    )

    use_b1 = bool(np.any(b1))
    use_b2 = bool(np.any(b2))
    key = (use_b1, use_b2)
    if key not in _program_cache:
        _program_cache[key] = _build_program(use_b1, use_b2)
    nc = _program_cache[key]

    a1_col = np.full((P, 1), a1, dtype=np.float32)
    a2_col = np.full((P, 1), a2, dtype=np.float32)

    in_maps = []
    for i in range(N_CORES):
        shard = x[i * TOK_PER_CORE : (i + 1) * TOK_PER_CORE]
        m = {
            "xt": np.ascontiguousarray(shard.T),
            "w1k": w1k,
            "w2k": w2k,
            "a1": a1_col,
            "a2": a2_col,
        }
        if use_b1:
            m["b1s"] = np.ascontiguousarray(b1.reshape(2, P).T)
        if use_b2:
            m["b2s"] = np.ascontiguousarray(b2.reshape(2, P).T)
        in_maps.append(m)
    return nc, in_maps


def kernel(x, w1, b1, alpha1, w2, b2, alpha2):
    nc, in_maps = _prepare(x, w1, b1, alpha1, w2, b2, alpha2)
    res = run_bass_kernel_spmd(nc, in_maps, list(range(N_CORES)))
    out = np.concatenate(
        [np.ascontiguousarray(res.results[i]["yt"].T.astype(np.float32)) for i in range(N_CORES)],
        axis=0,
    )
    return out


# revision 10
# speedup vs baseline: 1.7095x; 1.1037x over previous
"""TRN2 Bass kernel for the LSQ-quantized 2-layer MLP.

reference computation:
    wq1 = lsq_quant(w1, alpha1); wq2 = lsq_quant(w2, alpha2)   (tiny 256x256)
    h = relu(x @ wq1.T + b1)
    y = sigmoid(h @ wq2.T + b2)                                 x: [262144, 256] f32

Data-parallel over 8 NeuronCores (32768 tokens/core). Host-side prep per
shard: transpose x to channel-major (so the contraction dim lands on SBUF
partitions with plain DMAs — no on-chip transposes), and split the LSQ
quantization into integer levels k = round(clip(w/a, -8, 7)) (exactly
representable in bf16) and the scale a, which is folded into the activation
(relu(a*z+b1), sigmoid(a*z+b2)). Matmuls run in bf16 with fp32 PSUM
accumulation; the only precision loss is the bf16 rounding of x and h.

Everything is computed in the transposed (channel-major) domain:
    SWDGE cast-DMA xT tile (f32 DRAM -> bf16 SBUF)
    -> fc1: 4 matmuls (w1 chunks stationary, xT moving, N=512)  -> hT PSUM
    -> relu(a1*z [+ b1]) on DVE (ACT when b1 != 0)              -> bf16 SBUF
    -> fc2: 4 matmuls (w2 chunks stationary, hT moving, N=512)  -> yT PSUM
    -> sigmoid(a2*z [+ b2]) on ACT (b2 is per-partition here)   -> f32 SBUF
    -> DMA out yT (ACT queue; host un-transposes at gather)
"""

import ml_dtypes
import numpy as np

import concourse.bass as bass
import concourse.mybir as mybir
import concourse.tile as tile
from concourse import bacc
from concourse.bass import ts
from concourse.bass_utils import run_bass_kernel_spmd

N_CORES = 8
N_TOK = 262144
C = 256
TOK_PER_CORE = N_TOK // N_CORES  # 32768
T_MACRO = 512
N_MACROS = TOK_PER_CORE // T_MACRO  # 64
P = 128

F32 = mybir.dt.float32
F16 = mybir.dt.float16
BF16 = mybir.dt.bfloat16

_program_cache = {}


def _build_program(use_b1: bool, use_b2: bool):
    nc = bacc.Bacc("TRN2", target_bir_lowering=False, debug=False, num_devices=N_CORES)

    xt_d = nc.declare_dram_parameter("xt", [C, TOK_PER_CORE], BF16, isOutput=False)
    w1k_d = nc.declare_dram_parameter("w1k", [P, 2, C], BF16, isOutput=False)
    w2k_d = nc.declare_dram_parameter("w2k", [P, 2, C], BF16, isOutput=False)
    a1_d = nc.declare_dram_parameter("a1", [P, 1], F32, isOutput=False)
    a2_d = nc.declare_dram_parameter("a2", [P, 1], F32, isOutput=False)
    if use_b1:
        b1s_d = nc.declare_dram_parameter("b1s", [P, 2], F32, isOutput=False)
    if use_b2:
        b2s_d = nc.declare_dram_parameter("b2s", [P, 2], F32, isOutput=False)
    yt_d = nc.declare_dram_parameter("yt", [C, TOK_PER_CORE], F16, isOutput=True)

    # 1024-token super-macros: one 1 MiB load / store per pair of compute macros
    xt_v = xt_d.rearrange("(co ci) (m t) -> m ci co t", ci=P, t=2 * T_MACRO)
    yt_v = yt_d.rearrange("(co ci) (m t) -> m ci co t", ci=P, t=2 * T_MACRO)

    with tile.TileContext(nc) as tc:
        with (
            tc.tile_pool(name="const", bufs=1) as const_pool,
            tc.tile_pool(name="sb_xt", bufs=4) as sb_xt,
            tc.tile_pool(name="sb_ht", bufs=4) as sb_ht,
            tc.tile_pool(name="sb_yt", bufs=4) as sb_yt,
            tc.tile_pool(name="ps_h", bufs=4, space="PSUM") as ps_h,
            tc.tile_pool(name="ps_y", bufs=4, space="PSUM") as ps_y,
        ):
            w1k = const_pool.tile([P, 2, C], BF16)
            w2k = const_pool.tile([P, 2, C], BF16)
            nc.sync.dma_start(w1k[:], w1k_d[:])
            nc.sync.dma_start(w2k[:], w2k_d[:])
            a1 = const_pool.tile([P, 1], F32)
            a2 = const_pool.tile([P, 1], F32)
            nc.sync.dma_start(a1[:], a1_d[:])
            nc.sync.dma_start(a2[:], a2_d[:])
            if use_b1:
                b1s = const_pool.tile([P, 2], F32)
                nc.sync.dma_start(b1s[:], b1s_d[:])
            if use_b2:
                b2s = const_pool.tile([P, 2], F32)
                nc.sync.dma_start(b2s[:], b2s_d[:])

            for m in range(N_MACROS // 2):
                # x is pre-cast to bf16 on the host: plain HWDGE load, half the bytes
                xt = sb_xt.tile([P, 2, 2 * T_MACRO], BF16, tag="xt")
                nc.sync.dma_start(xt[:], xt_v[m])

                yt = sb_yt.tile([P, 2, 2 * T_MACRO], F16, tag="yt")
                for s in range(2):
                    tok = ts(s, T_MACRO)
                    # fc1: hT[j_chunk] = sum_c w1k[:,c,jchunk].T @ xT[:,c,:]
                    ht = sb_ht.tile([P, 2, T_MACRO], BF16, tag="ht")
                    for j in range(2):
                        pht = ps_h.tile([P, T_MACRO], F32, tag="pht")
                        for c in range(2):
                            nc.tensor.matmul(
                                pht[:],
                                w1k[:, c, ts(j, P)],
                                xt[:, c, tok],
                                start=(c == 0),
                                stop=(c == 1),
                            )
                        if use_b1:
                            nc.scalar.activation(
                                ht[:, j, :],
                                pht[:],
                                mybir.ActivationFunctionType.Relu,
                                bias=b1s[:, j : j + 1],
                                scale=a1[:],
                            )
                        else:
                            # relu(a1*z) on DVE: (z * a1) max 0
                            nc.vector.tensor_scalar(
                                ht[:, j, :],
                                pht[:],
                                a1[:],
                                0.0,
                                mybir.AluOpType.mult,
                                mybir.AluOpType.max,
                            )

                    # fc2: yT[j_chunk] = sum_c w2k[:,c,jchunk].T @ hT[:,c,:]
                    for j in range(2):
                        pyt = ps_y.tile([P, T_MACRO], F32, tag="pyt")
                        for c in range(2):
                            nc.tensor.matmul(
                                pyt[:],
                                w2k[:, c, ts(j, P)],
                                ht[:, c, :],
                                start=(c == 0),
                                stop=(c == 1),
                            )
                        nc.scalar.activation(
                            yt[:, j, tok],
                            pyt[:],
                            mybir.ActivationFunctionType.Sigmoid,
                            bias=b2s[:, j : j + 1] if use_b2 else 0.0,
                            scale=a2[:],
                        )
                nc.gpsimd.dma_start(yt_v[m], yt[:])

    nc.compile()
    return nc


def _quantize_lsq_int(w: np.ndarray, alpha) -> tuple[np.ndarray, np.float32]:
    """Integer LSQ levels k = round(clip(w/a, -8, 7)) and effective scale a,
    replicating the reference forward numerics in np float32."""
    one = np.float32(1.0)
    g = one / np.sqrt(np.float32(w.size * 7))
    alpha = np.float32(alpha)
    a = np.float32(alpha * g) + np.float32(alpha * np.float32(one - g))
    t = np.clip((w / a).astype(np.float32), np.float32(-8.0), np.float32(7.0))
    r = (np.round(t) - t).astype(np.float32)
    q = (t + r).astype(np.float32)  # integer levels in [-8, 7]
    return q, a


def _prepare(x, w1, b1, alpha1, w2, b2, alpha2):
    x = np.asarray(x, dtype=np.float32)
    w1 = np.asarray(w1, dtype=np.float32)
    w2 = np.asarray(w2, dtype=np.float32)
    b1 = np.asarray(b1, dtype=np.float32)
    b2 = np.asarray(b2, dtype=np.float32)

    k1, a1 = _quantize_lsq_int(w1, alpha1)
    k2, a2 = _quantize_lsq_int(w2, alpha2)

    # lhsT layouts: w1k[ci, co, j] = k1[j, co*128+ci]
    w1k = np.ascontiguousarray(k1.T.reshape(2, P, C).transpose(1, 0, 2)).astype(
        ml_dtypes.bfloat16
    )
    w2k = np.ascontiguousarray(k2.T.reshape(2, P, C).transpose(1, 0, 2)).astype(
        ml_dtypes.bfloat16
    )

    use_b1 = bool(np.any(b1))
    use_b2 = bool(np.any(b2))
    key = (use_b1, use_b2)
    if key not in _program_cache:
        _program_cache[key] = _build_program(use_b1, use_b2)
    nc = _program_cache[key]

    a1_col = np.full((P, 1), a1, dtype=np.float32)
    a2_col = np.full((P, 1), a2, dtype=np.float32)

    in_maps = []
    for i in range(N_CORES):
        shard = x[i * TOK_PER_CORE : (i + 1) * TOK_PER_CORE]
        m = {
            "xt": np.ascontiguousarray(shard.T).astype(ml_dtypes.bfloat16),
            "w1k": w1k,
            "w2k": w2k,
            "a1": a1_col,
            "a2": a2_col,
        }
        if use_b1:
            m["b1s"] = np.ascontiguousarray(b1.reshape(2, P).T)
        if use_b2:
            m["b2s"] = np.ascontiguousarray(b2.reshape(2, P).T)
        in_maps.append(m)
    return nc, in_maps


def kernel(x, w1, b1, alpha1, w2, b2, alpha2):
    nc, in_maps = _prepare(x, w1, b1, alpha1, w2, b2, alpha2)
    res = run_bass_kernel_spmd(nc, in_maps, list(range(N_CORES)))
    out = np.concatenate(
        [np.ascontiguousarray(res.results[i]["yt"].T.astype(np.float32)) for i in range(N_CORES)],
        axis=0,
    )
    return out
